# revision 30
# baseline (speedup 1.0000x reference)
"""CNN+RNN fused Trainium2 kernel, 8-core data parallel (batch 8192 -> 1024/core).

Model: Conv2d(1->16, 3x3, pad=1)+bias+ReLU -> MaxPool2d(2) -> flatten ->
Linear(3136->256)+b_in -> r=relu(E0) -> 9x r=relu(r@W + b_in + r) ->
Linear(256->10)+b_out.

Per-core strategy (all matmul operands bf16 = 1 PE cycle/row, fp32 PSUM):
- Host preps a pixel-major, zero-row-padded image matrix xT_pad [840, B] in
  bf16 and a banded conv-as-matmul operator A [112, 896] whose 8 M-chunks
  are grouped by 2x2 pool class, plus pre-permuted bf16 blocks of W_in
  (matching the pooled-tile layout), W, W_out^T, and fp32 bias vectors.
- Conv: per 2-row block, 4 matmuls (one per pool class) from a 4-row halo
  tile into PSUM. relu+bias+maxpool is a split reduction: ScalarE relu-drains
  2 (or 3) classes, VectorE does fused (psum+bias) max (drain) via
  scalar_tensor_tensor plus a bf16 2x-mode tensor_max combine, writing
  pooled activations p^T [3136, B] bf16 directly in W_in's rhs layout.
- W_in accumulates 28 K-tiles into PSUM per output chunk; bias+relu fuse on
  ScalarE (bf16 copy for the PE) and VectorE (fp32 master).
- Recurrence keeps r in fp32 (rf) + bf16 (rb): PE computes the tiny r@W term
  in bf16; VectorE adds b_in + rf in fp32 (scalar_tensor_tensor); relu splits
  across ScalarE/VectorE. This avoids compounding bf16 rounding of r (folding
  +I into bf16 W costs ~1.5% error; this scheme measures ~0.45%).
- W_out runs h-major ([10, B] output, host transposes after gather).
Engine balance (cost model): DVE ~118us, ACT ~117us, PE ~88us, span ~140.7us.
"""
import sys
sys.path.insert(0, "/opt/trn_rl_repo")
from contextlib import ExitStack

import numpy as np
import ml_dtypes

import concourse.bacc as bacc
import concourse.tile as tile
from concourse import mybir
from concourse.bass_utils import run_bass_kernel_spmd

BF16 = ml_dtypes.bfloat16
NCORES = 8
B = 8192
BS = B // NCORES          # 1024 per core
C = 16
H = 256
OUT = 10
IMG = 28
NBLK = 14                 # row-pair blocks
HALO = 112                # 4 image rows
NPOOL = 28                # pooled K-tiles of 112 rows (28*112 = 3136)

_CACHE = {}


def _build_amat(conv_w):
    """A [112, 896]: conv-as-matmul for one 2-row output block.

    Column m = mc*112 + q, mc = cls*2 + j, cls = a*2 + b (a=row-in-pair,
    b=col parity), channel c = 8j + q//14, pooled col jp = q%14.
    Input rows = halo pixels (4 image rows, row-major).
    """
    A = np.zeros((112, 8 * 112), np.float32)
    for mc in range(8):
        cls, j = mc // 2, mc % 2
        a, bpar = cls // 2, cls % 2
        for q in range(112):
            c = 8 * j + q // 14
            jp = q % 14
            m = mc * 112 + q
            cc = 2 * jp + bpar
            for di in range(3):
                for dj in range(3):
                    icol = cc - 1 + dj
                    if 0 <= icol < IMG:
                        A[(a + di) * IMG + icol, m] += conv_w[c, 0, di, dj]
    return A


def _build_wg(W_in):
    """Wg [112, 28*256]: W_in^T blocked to match pooled-tile layout.

    Pooled tile t = 2s+j holds rows q -> (c = 8j + q//14, i'=s, jp = q%14),
    i.e. W_in column c*196 + s*14 + jp.
    """
    Wg = np.zeros((112, NPOOL * H), np.float32)
    q = np.arange(112)
    for t in range(NPOOL):
        s, j = t // 2, t % 2
        cols = (8 * j + q // 14) * 196 + s * 14 + (q % 14)
        Wg[:, t * H:(t + 1) * H] = W_in[:, cols].T
    return Wg


def _build_graph():
    nc = bacc.Bacc("TRN2", target_bir_lowering=False, debug=False)
    f32, bf16 = mybir.dt.float32, mybir.dt.bfloat16
    AL = mybir.AluOpType
    RELU = mybir.ActivationFunctionType.Relu

    xt = nc.dram_tensor("xt", [840, BS], bf16, kind="ExternalInput").ap()
    amat = nc.dram_tensor("amat", [HALO, 896], bf16, kind="ExternalInput").ap()
    wg = nc.dram_tensor("wg", [112, NPOOL * H], bf16, kind="ExternalInput").ap()
    wrec = nc.dram_tensor("wrec", [128, 512], bf16, kind="ExternalInput").ap()
    wout = nc.dram_tensor("wout", [128, 2 * OUT], bf16, kind="ExternalInput").ap()
    binp = nc.dram_tensor("binp", [128, 2], f32, kind="ExternalInput").ap()
    bconv = nc.dram_tensor("bconv", [112, 2], f32, kind="ExternalInput").ap()
    boutp = nc.dram_tensor("boutp", [OUT, 1], f32, kind="ExternalInput").ap()
    out = nc.dram_tensor("out", [OUT, BS], f32, kind="ExternalOutput").ap()

    with tile.TileContext(nc) as tc, ExitStack() as ctx:
        const = ctx.enter_context(tc.tile_pool(name="const", bufs=1))
        halo_p = ctx.enter_context(tc.tile_pool(name="halo", bufs=4))
        cpsum = ctx.enter_context(tc.tile_pool(name="cpsum", bufs=4, space="PSUM"))
        apsum = ctx.enter_context(tc.tile_pool(name="apsum", bufs=1, space="PSUM"))
        tmp = ctx.enter_context(tc.tile_pool(name="tmp", bufs=4))
        pooled_p = ctx.enter_context(tc.tile_pool(name="pooled", bufs=1))
        rp = ctx.enter_context(tc.tile_pool(name="rp", bufs=3))
        outp = ctx.enter_context(tc.tile_pool(name="outp", bufs=2))

        # Dummy relu at t=0: pulls the one-time ACT function-table load
        # (~2.7us) into the DMA startup window instead of delaying the
        # first conv drain.
        warm = const.tile([128, 16], f32, name="warm")
        nc.gpsimd.memset(warm[:], 0.0)
        nc.scalar.activation(warm[:], warm[:], RELU)
        t_amat = const.tile([HALO, 896], bf16)
        nc.sync.dma_start(t_amat[:], amat[:])
        t_bconv = const.tile([112, 2], f32)
        nc.sync.dma_start(t_bconv[:], bconv[:])
        t_bin = const.tile([128, 2], f32)
        nc.sync.dma_start(t_bin[:], binp[:])
        # first few halo loads ahead of the big weight loads so conv starts asap
        halos = {}
        for s in range(3):
            halos[s] = halo_p.tile([HALO, BS], bf16, name=f"halo{s}", tag="halo")
            if s == 0:
                # split: the first conv unit only needs the n=0 half
                nc.sync.dma_start(halos[s][:, 0:512], xt[56 * s:56 * s + 112, 0:512])
                nc.sync.dma_start(halos[s][:, 512:1024], xt[56 * s:56 * s + 112, 512:1024])
            else:
                nc.sync.dma_start(halos[s][:], xt[56 * s:56 * s + 112, :])
        t_wg = const.tile([112, NPOOL * H], bf16)
        nc.sync.dma_start(t_wg[:], wg[:])
        t_wrec = const.tile([128, 512], bf16)
        nc.sync.dma_start(t_wrec[:], wrec[:])
        t_wout = const.tile([128, 2 * OUT], bf16)
        nc.sync.dma_start(t_wout[:], wout[:])
        t_bout = const.tile([OUT, 1], f32)
        nc.sync.dma_start(t_bout[:], boutp[:])

        pooled = []
        for t in range(NPOOL):
            pt = pooled_p.tile([112, BS], bf16, name=f"pooled{t}", tag=f"pooled{t}")
            pooled.append(pt)

        # ---- conv + relu + maxpool ----
        for s in range(NBLK):
            if s in halos:
                halo = halos[s]
            else:
                halo = halo_p.tile([HALO, BS], bf16, name=f"halo{s}", tag="halo")
                nc.sync.dma_start(halo[:], xt[56 * s:56 * s + 112, :])
            for n in range(2):
                nsl = slice(n * 512, (n + 1) * 512)
                for j in range(2):
                    ps = []
                    for cls in range(4):
                        mc = cls * 2 + j
                        p = cpsum.tile([112, 512], f32, name=f"cv{s}_{n}_{j}_{cls}",
                                       tag="cv")
                        nc.tensor.matmul(p[:], t_amat[:, mc * 112:(mc + 1) * 112],
                                         halo[:, nsl], start=True, stop=True)
                        ps.append(p)
                    bcj = t_bconv[:, j:j + 1]
                    unit = (s * 2 + n) * 2 + j
                    if unit % 8 < 3:
                        # 3 ACT drains, 1 DVE STT: offloads DVE (the conv
                        # bottleneck) onto ScalarE for a fraction of the units
                        t0 = tmp.tile([112, 512], bf16, name=f"t0_{s}_{n}_{j}", tag="t0b")
                        nc.scalar.activation(t0[:], ps[0][:], RELU, bias=bcj)
                        u1 = tmp.tile([112, 512], bf16, name=f"u1_{s}_{n}_{j}", tag="u1b")
                        nc.scalar.activation(u1[:], ps[1][:], RELU, bias=bcj)
                        t2 = tmp.tile([112, 512], f32, name=f"t2_{s}_{n}_{j}", tag="t2")
                        nc.scalar.activation(t2[:], ps[2][:], RELU, bias=bcj)
                        m1 = tmp.tile([112, 512], bf16, name=f"m1_{s}_{n}_{j}", tag="m1")
                        nc.vector.tensor_max(m1[:], t0[:], u1[:])
                        m2 = tmp.tile([112, 512], bf16, name=f"m2_{s}_{n}_{j}", tag="m2")
                        nc.vector.scalar_tensor_tensor(m2[:], ps[3][:], bcj, t2[:],
                                                       op0=AL.add, op1=AL.max)
                    else:
                        t0 = tmp.tile([112, 512], f32, name=f"t0_{s}_{n}_{j}", tag="t0")
                        nc.scalar.activation(t0[:], ps[0][:], RELU, bias=bcj)
                        t2 = tmp.tile([112, 512], f32, name=f"t2_{s}_{n}_{j}", tag="t2")
                        nc.scalar.activation(t2[:], ps[2][:], RELU, bias=bcj)
                        m1 = tmp.tile([112, 512], bf16, name=f"m1_{s}_{n}_{j}", tag="m1")
                        nc.vector.scalar_tensor_tensor(m1[:], ps[1][:], bcj, t0[:],
                                                       op0=AL.add, op1=AL.max)
                        m2 = tmp.tile([112, 512], bf16, name=f"m2_{s}_{n}_{j}", tag="m2")
                        nc.vector.scalar_tensor_tensor(m2[:], ps[3][:], bcj, t2[:],
                                                       op0=AL.add, op1=AL.max)
                    # m1, m2 are both >= 0 (each max includes a relu'd arm),
                    # so max(m1, m2) == relu(max of all four biased values).
                    nc.vector.tensor_max(pooled[2 * s + j][:, nsl], m1[:], m2[:])

        # ---- W_in + bias + relu -> r0 (bf16 rb for the PE, fp32 rf master) ----
        rb = {}
        rf = {}
        for mch in range(2):
            rb[(0, mch)] = rp.tile([128, BS], bf16, name=f"rb0_{mch}",
                                   tag=f"rb{mch}")
            rf[(0, mch)] = rp.tile([128, BS], f32, name=f"rf0_{mch}",
                                   tag=f"rf{mch}")
        for mch in range(2):
            bia = t_bin[:, mch:mch + 1]
            for n in range(2):
                nsl = slice(n * 512, (n + 1) * 512)
                e0 = apsum.tile([128, 512], f32, name=f"e0_{mch}_{n}",
                                tag=f"acc{mch * 2 + n}")
                for t in range(NPOOL):
                    nc.tensor.matmul(
                        e0[:], t_wg[:, t * H + mch * 128: t * H + mch * 128 + 128],
                        pooled[t][:, nsl], start=(t == 0), stop=(t == NPOOL - 1))
                nc.scalar.activation(rb[(0, mch)][:, nsl], e0[:], RELU, bias=bia)
                nc.vector.tensor_scalar(rf[(0, mch)][:, nsl], e0[:], bia, 0.0,
                                        op0=AL.add, op1=AL.max)

        # ---- 9 recurrent steps: r' = relu(r @ W + b_in + r), fp32 master ----
        for k in range(1, 10):
            for mch in range(2):
                rb[(k, mch)] = rp.tile([128, BS], bf16, name=f"rb{k}_{mch}",
                                       tag=f"rb{mch}")
                if k < 9:
                    rf[(k, mch)] = rp.tile([128, BS], f32, name=f"rf{k}_{mch}",
                                           tag=f"rf{mch}")
            for n in range(2):
                nsl = slice(n * 512, (n + 1) * 512)
                for mch in range(2):
                    bia = t_bin[:, mch:mch + 1]
                    pr = apsum.tile([128, 512], f32, name=f"pr{k}_{mch}_{n}",
                                    tag=f"acc{mch * 2 + n}")
                    for kc in range(2):
                        nc.tensor.matmul(
                            pr[:], t_wrec[:, (kc * 2 + mch) * 128:(kc * 2 + mch) * 128 + 128],
                            rb[(k - 1, kc)][:, nsl], start=(kc == 0), stop=(kc == 1))
                    pre = tmp.tile([128, 512], f32, name=f"pre{k}_{mch}_{n}",
                                   tag=f"pre{mch}")
                    nc.vector.scalar_tensor_tensor(
                        pre[:], pr[:], bia, rf[(k - 1, mch)][:, nsl],
                        op0=AL.add, op1=AL.add)
                    nc.scalar.activation(rb[(k, mch)][:, nsl], pre[:], RELU)
                    if k < 9:
                        if (k + mch + n) % 2 == 0:
                            nc.scalar.activation(rf[(k, mch)][:, nsl], pre[:], RELU)
                        else:
                            nc.vector.tensor_scalar_max(rf[(k, mch)][:, nsl], pre[:], 0.0)

        # ---- W_out + b_out (h-major output; host transposes) ----
        for n in range(2):
            nsl = slice(n * 512, (n + 1) * 512)
            po = apsum.tile([OUT, 512], f32, name=f"po{n}", tag=f"acc{n}")
            for kc in range(2):
                nc.tensor.matmul(po[:], t_wout[:, kc * OUT:(kc + 1) * OUT],
                                 rb[(9, kc)][:, nsl],
                                 start=(kc == 0), stop=(kc == 1))
            ot = outp.tile([OUT, 512], f32, name=f"ot{n}", tag="ot")
            nc.vector.tensor_scalar_add(ot[:], po[:], t_bout[:, 0:1])
            nc.sync.dma_start(out[:, nsl], ot[:])

    nc.compile()
    return nc


def _prep_host(inputs):
    x = np.asarray(inputs["x"], np.float32).reshape(B, 784)
    conv_w = np.asarray(inputs["conv_w"], np.float32)
    conv_b = np.asarray(inputs["conv_b"], np.float32)
    W_in = np.asarray(inputs["W_in"], np.float32)
    b_in = np.asarray(inputs["b_in"], np.float32)
    W_out = np.asarray(inputs["W_out"], np.float32)
    b_out = np.asarray(inputs["b_out"], np.float32)
    W = np.asarray(inputs["W"], np.float32)

    xT = np.zeros((840, B), np.float32)
    xT[28:812, :] = x.T
    xT = xT.astype(BF16)

    A = _build_amat(conv_w).astype(BF16)
    q = np.arange(112)
    bconv = np.stack([conv_b[8 * j + q // 14] for j in range(2)], axis=1).astype(np.float32)
    Wg = _build_wg(W_in).astype(BF16)

    # wrec [128, 4*128]: blocks of W. The +r term is added in fp32 on DVE
    # (folding +I into bf16 W compounds r's rounding each step: ~1.5% error).
    wrec = np.zeros((128, 512), np.float32)
    for kc in range(2):
        for mch in range(2):
            wrec[:, (kc * 2 + mch) * 128:(kc * 2 + mch) * 128 + 128] = \
                W[kc * 128:(kc + 1) * 128, mch * 128:(mch + 1) * 128]
    wrec = wrec.astype(BF16)

    woutb = np.zeros((128, 2 * OUT), np.float32)
    for kc in range(2):
        woutb[:, kc * OUT:(kc + 1) * OUT] = W_out[:, kc * 128:(kc + 1) * 128].T
    woutb = woutb.astype(BF16)

    binp = b_in.reshape(2, 128).T.copy()
    boutp = b_out.reshape(OUT, 1).astype(np.float32)

    common = {"amat": A, "wg": Wg, "wrec": wrec, "wout": woutb,
              "binp": binp, "boutp": boutp, "bconv": bconv}
    in_maps = []
    for c in range(NCORES):
        m = dict(common)
        m["xt"] = np.ascontiguousarray(xT[:, c * BS:(c + 1) * BS])
        in_maps.append(m)
    return in_maps


def kernel(**inputs):
    if "nc" not in _CACHE:
        _CACHE["nc"] = _build_graph()
    nc = _CACHE["nc"]
    in_maps = _prep_host(inputs)
    res = run_bass_kernel_spmd(nc, in_maps, core_ids=list(range(NCORES)))
    _CACHE["last_result"] = res
    outs = [res.results[c]["out"].T for c in range(NCORES)]
    return np.ascontiguousarray(np.concatenate(outs, axis=0)).astype(np.float32)
